# revision 10
# baseline (speedup 1.0000x reference)
"""Contrastive loss (video/audio) Trainium2 Bass kernel.

Full inputs: video [64,512,512] f32, audio [64,512,512] f32, mask [64,512] i32.
Data-parallel over batch: 8 cores x 8 batch elements. Each core computes its
partial loss sum on device; host adds the 8 scalars and divides by B.

Per-core pipeline (v3):
  argmax(mask) via score trick -> anchor row ids -> narrow indirect gather
  [8,512] (raw anchors) -> per-b PE outer-product broadcast using an 8x8
  selector matrix (eye column broadcast as lhsT) -> PSUM [128,512] anchors.
  Main loop per b,c-chunk: r = ACT Square+accum; s_raw = DVE STT(mult,mult)
  with accum.  All normalization (sqrt/reciprocal), pos, exp, log and the
  final combine happen after the main loop; per-b anchor inverse norms are
  broadcast to 128 partitions with tiny PE outer-products and applied with a
  stride-0 free-dim AP.  One scalar partial sum is DMA'd out per core.
"""

import numpy as np
from contextlib import ExitStack

import concourse.bass as bass
import concourse.tile as tile
from concourse import mybir
from concourse.bass_utils import run_bass_kernel_spmd

F32 = mybir.dt.float32
BF16 = mybir.dt.bfloat16
I32 = mybir.dt.int32
AF = mybir.ActivationFunctionType
OP = mybir.AluOpType
AX = mybir.AxisListType

B, T, D = 64, 512, 512
NCORES = 8
BL = B // NCORES          # 8 batch elements per core
P = 128                   # partitions
C = T // P                # 4 T-chunks per matrix
TEMP = 0.07
USE_BF16 = True           # ship bf16 data; accumulate in fp32
R_ON_DVE = 26             # of the 64 r-square chunks, this many run on DVE


def build_kernel(ctx: ExitStack, tc: tile.TileContext, video, audio, mask, out):
    nc = tc.nc
    DT = BF16 if USE_BF16 else F32

    persist = ctx.enter_context(tc.tile_pool(name="persist", bufs=1))
    data = ctx.enter_context(tc.tile_pool(name="data", bufs=3))
    scr = ctx.enter_context(tc.tile_pool(name="scr", bufs=2))
    psum = ctx.enter_context(tc.tile_pool(name="psum", bufs=2, space="PSUM"))
    psum1 = ctx.enter_context(tc.tile_pool(name="psum1", bufs=1, space="PSUM"))
    dram = ctx.enter_context(tc.tile_pool(name="dram", bufs=1, space="DRAM"))

    vrows = video.rearrange("b t d -> (b t) d")     # [4096, 512]
    arows = audio.rearrange("b t d -> (b t) d")

    # ---------------- argmax(mask) -> anchor row ids -----------------------
    mask_i = persist.tile([BL, T], I32, tag="mask_i")
    nc.sync.dma_start(mask_i[:], mask[:, :])
    mask_f = persist.tile([BL, T], F32, tag="mask_f")
    nc.vector.tensor_copy(mask_f[:], mask_i[:])
    iota_i = persist.tile([BL, T], I32, tag="iota_i")
    nc.gpsimd.iota(iota_i[:], pattern=[[1, T]], base=0, channel_multiplier=0)
    iota_f = persist.tile([BL, T], F32, tag="iota_f")
    nc.vector.tensor_copy(iota_f[:], iota_i[:])
    score = persist.tile([BL, T], F32, tag="score")
    nc.vector.scalar_tensor_tensor(
        out=score[:], in0=mask_f[:], scalar=1024.0, in1=iota_f[:],
        op0=OP.mult, op1=OP.subtract)
    maxs = persist.tile([BL, 1], F32, tag="maxs")
    nc.vector.reduce_max(maxs[:], score[:], axis=AX.X)
    idx_f = persist.tile([BL, 1], F32, tag="idx_f")
    nc.vector.tensor_scalar(
        out=idx_f[:], in0=maxs[:], scalar1=-1.0, scalar2=1024.0,
        op0=OP.mult, op1=OP.add)
    nc.vector.tensor_scalar_min(idx_f[:], idx_f[:], 511.0)
    brow_i = persist.tile([BL, 1], I32, tag="brow_i")
    nc.gpsimd.iota(brow_i[:], pattern=[[1, 1]], base=0, channel_multiplier=T)
    brow_f = persist.tile([BL, 1], F32, tag="brow_f")
    nc.vector.tensor_copy(brow_f[:], brow_i[:])
    row_f = persist.tile([BL, 1], F32, tag="row_f")
    nc.vector.tensor_add(row_f[:], idx_f[:], brow_f[:])
    row_i = persist.tile([BL, 1], I32, tag="row_i")
    nc.vector.tensor_copy(row_i[:], row_f[:])

    # 8x8 selector: eye8[k,m] = (k == m), as f32
    eyei = persist.tile([BL, BL], I32, tag="eyei")
    nc.gpsimd.iota(eyei[:], pattern=[[1, BL]], base=0, channel_multiplier=-1)
    eyez = persist.tile([BL, BL], I32, tag="eyez")
    nc.vector.tensor_scalar(out=eyez[:], in0=eyei[:], scalar1=0,
                            scalar2=None, op0=OP.is_equal)
    eyef = persist.tile([BL, BL], DT, tag="eyef")
    nc.vector.tensor_copy(eyef[:], eyez[:])

    # ---------------- narrow anchor gathers: [8, 512] (raw) ----------------
    anc_v = persist.tile([BL, D], DT, tag="anc_v")
    nc.gpsimd.indirect_dma_start(
        out=anc_v[:], out_offset=None, in_=vrows[:],
        in_offset=bass.IndirectOffsetOnAxis(ap=row_i[:, :1], axis=0))
    anc_a = persist.tile([BL, D], DT, tag="anc_a")
    nc.gpsimd.indirect_dma_start(
        out=anc_a[:], out_offset=None, in_=arows[:],
        in_offset=bass.IndirectOffsetOnAxis(ap=row_i[:, :1], axis=0))

    # ---------------- main loop ---------------------------------------------
    rv_t = persist.tile([P, BL * C], F32, tag="rv_t")   # ||video_t||^2
    ra_t = persist.tile([P, BL * C], F32, tag="ra_t")   # ||audio_t||^2
    sa_t = persist.tile([P, BL * C], F32, tag="sa_t")   # video_t . anc_a(raw)
    sv_t = persist.tile([P, BL * C], F32, tag="sv_t")   # audio_t . anc_v(raw)

    # contiguous-row tiling: t = 4*p + c -> one 8KB descriptor per partition
    vid_r = video.rearrange("b (p c) d -> b p c d", p=P)   # [8,128,4,512]
    aud_r = audio.rearrange("b (p c) d -> b p c d", p=P)

    for b in range(BL):
        vt = data.tile([P, C * D], DT, tag="vid")
        nc.sync.dma_start(vt[:].rearrange("p (c d) -> p c d", d=D), vid_r[b])
        at = data.tile([P, C * D], DT, tag="aud")
        nc.sync.dma_start(at[:].rearrange("p (c d) -> p c d", d=D), aud_r[b])
        # anchor_b broadcast to all partitions: sel_b[8,128].T @ anchors[8,512]
        sel = eyef[:, b:b + 1].to_broadcast([BL, P])
        abc_ps = psum.tile([P, D], F32, tag="abc_ps")
        nc.tensor.matmul(out=abc_ps[:], lhsT=sel, rhs=anc_a[:],
                         start=True, stop=True)
        abc = data.tile([P, D], DT, tag="abc")
        nc.scalar.copy(abc[:], abc_ps[:])
        vbc_ps = psum.tile([P, D], F32, tag="vbc_ps")
        nc.tensor.matmul(out=vbc_ps[:], lhsT=sel, rhs=anc_v[:],
                         start=True, stop=True)
        vbc = data.tile([P, D], DT, tag="vbc")
        nc.scalar.copy(vbc[:], vbc_ps[:])
        for c in range(C):
            col = b * C + c
            vch = vt[:, c * D:(c + 1) * D]
            ach = at[:, c * D:(c + 1) * D]
            r_dve = (2 * col) % 64 < R_ON_DVE
            r1 = scr.tile([P, D], DT, tag="r1")
            if r_dve:
                nc.vector.scalar_tensor_tensor(
                    out=r1[:], in0=vch, scalar=1.0, in1=vch,
                    op0=OP.mult, op1=OP.mult,
                    accum_out=rv_t[:, col:col + 1])
            else:
                nc.scalar.activation(r1[:], vch, AF.Square,
                                     accum_out=rv_t[:, col:col + 1])
            r2 = scr.tile([P, D], DT, tag="r2")
            if (2 * col + 1) % 64 < R_ON_DVE:
                nc.vector.scalar_tensor_tensor(
                    out=r2[:], in0=ach, scalar=1.0, in1=ach,
                    op0=OP.mult, op1=OP.mult,
                    accum_out=ra_t[:, col:col + 1])
            else:
                nc.scalar.activation(r2[:], ach, AF.Square,
                                     accum_out=ra_t[:, col:col + 1])
            s1 = scr.tile([P, D], DT, tag="s1")
            nc.vector.scalar_tensor_tensor(
                out=s1[:], in0=vch, scalar=1.0, in1=abc[:],
                op0=OP.mult, op1=OP.mult, accum_out=sa_t[:, col:col + 1])
            s2 = scr.tile([P, D], DT, tag="s2")
            nc.vector.scalar_tensor_tensor(
                out=s2[:], in0=ach, scalar=1.0, in1=vbc[:],
                op0=OP.mult, op1=OP.mult, accum_out=sv_t[:, col:col + 1])

    # ------- anchor norms, pos (computed late, off the critical path) ------
    nsc_v = persist.tile([BL, D], F32, tag="nsc_v")
    ran_v = persist.tile([BL, 1], F32, tag="ran_v")
    nc.vector.scalar_tensor_tensor(
        out=nsc_v[:], in0=anc_v[:], scalar=1.0, in1=anc_v[:],
        op0=OP.mult, op1=OP.mult, accum_out=ran_v[:])
    nsc_a = persist.tile([BL, D], F32, tag="nsc_a")
    ran_a = persist.tile([BL, 1], F32, tag="ran_a")
    nc.vector.scalar_tensor_tensor(
        out=nsc_a[:], in0=anc_a[:], scalar=1.0, in1=anc_a[:],
        op0=OP.mult, op1=OP.mult, accum_out=ran_a[:])
    pd_scr = persist.tile([BL, D], F32, tag="pd_scr")
    posd = persist.tile([BL, 1], F32, tag="posd")
    nc.vector.scalar_tensor_tensor(
        out=pd_scr[:], in0=anc_v[:], scalar=1.0, in1=anc_a[:],
        op0=OP.mult, op1=OP.mult, accum_out=posd[:])
    # round-trip (ran_a, ran_v, posd) to a partition-0 row [1, 24]
    pk = persist.tile([BL, 3], F32, tag="pk")
    nc.vector.tensor_copy(pk[:, 0:1], ran_a[:])
    nc.vector.tensor_copy(pk[:, 1:2], ran_v[:])
    nc.vector.tensor_copy(pk[:, 2:3], posd[:])
    d_pk = dram.tile([BL, 3], F32, tag="d_pk")
    nc.sync.dma_start(d_pk[:], pk[:])
    pkr = persist.tile([1, BL * 3], F32, tag="pkr")
    nc.sync.dma_start(pkr[:], d_pk[:].rearrange("a b -> (a b)"))
    pkr3 = pkr[:].rearrange("p (a b) -> p a b", b=3)
    raa_row = persist.tile([1, BL], F32, tag="raa_row")
    nc.vector.tensor_copy(raa_row[:], pkr3[:, :, 0:1].rearrange("p a b -> p (a b)"))
    rav_row = persist.tile([1, BL], F32, tag="rav_row")
    nc.vector.tensor_copy(rav_row[:], pkr3[:, :, 1:2].rearrange("p a b -> p (a b)"))
    pod_row = persist.tile([1, BL], F32, tag="pod_row")
    nc.vector.tensor_copy(pod_row[:], pkr3[:, :, 2:3].rearrange("p a b -> p (a b)"))
    # inv rows: 1/(TEMP*sqrt(r))
    sq_ra = persist.tile([1, BL], F32, tag="sq_ra")
    nc.scalar.activation(sq_ra[:], raa_row[:], AF.Sqrt, scale=TEMP * TEMP)
    inva_row = persist.tile([1, BL], F32, tag="inva_row")
    nc.vector.reciprocal(inva_row[:], sq_ra[:])
    sq_rv = persist.tile([1, BL], F32, tag="sq_rv")
    nc.scalar.activation(sq_rv[:], rav_row[:], AF.Sqrt, scale=TEMP * TEMP)
    invv_row = persist.tile([1, BL], F32, tag="invv_row")
    nc.vector.reciprocal(invv_row[:], sq_rv[:])
    # pos = posd * inv_a * inv_v * TEMP   on [1,8]
    pos_row = persist.tile([1, BL], F32, tag="pos_row")
    nc.vector.tensor_tensor(pos_row[:], pod_row[:], inva_row[:], op=OP.mult)
    nc.vector.tensor_tensor(pos_row[:], pos_row[:], invv_row[:], op=OP.mult)
    nc.vector.tensor_scalar_mul(pos_row[:], pos_row[:], TEMP)
    # broadcast inv rows to [128, 8] via PE outer
    ones_row = persist.tile([1, P], F32, tag="ones_row")
    nc.vector.memset(ones_row[:], 1.0)
    ones_col = persist.tile([P, 1], F32, tag="ones_col")
    nc.vector.memset(ones_col[:], 1.0)
    inv_bc = psum1.tile([P, 2 * BL], F32, tag="inv_bc")
    nc.tensor.matmul(out=inv_bc[:, 0:BL], lhsT=ones_row[:], rhs=inva_row[:],
                     start=True, stop=True)
    nc.tensor.matmul(out=inv_bc[:, BL:2 * BL], lhsT=ones_row[:],
                     rhs=invv_row[:], start=True, stop=True)
    inva_bc = inv_bc[:, 0:BL]
    invv_bc = inv_bc[:, BL:2 * BL]

    # ---------------- post: scale, exp, reduce, combine --------------------
    srt_v = persist.tile([P, BL * C], F32, tag="srt_v")
    nc.scalar.activation(srt_v[:], rv_t[:], AF.Sqrt)
    irt_v = persist.tile([P, BL * C], F32, tag="irt_v")
    nc.vector.reciprocal(irt_v[:], srt_v[:])
    srt_a = persist.tile([P, BL * C], F32, tag="srt_a")
    nc.scalar.activation(srt_a[:], ra_t[:], AF.Sqrt)
    irt_a = persist.tile([P, BL * C], F32, tag="irt_a")
    nc.vector.reciprocal(irt_a[:], srt_a[:])

    # combined scale: irt * anchor_inv(b)  (stride-0 broadcast over c)
    cmb_a = persist.tile([P, BL, C], F32, tag="cmb_a")
    nc.vector.tensor_tensor(
        cmb_a[:], irt_v[:].rearrange("p (a b) -> p a b", b=C),
        inva_bc.to_broadcast([P, BL, C]),
        op=OP.mult)
    cmb_v = persist.tile([P, BL, C], F32, tag="cmb_v")
    nc.vector.tensor_tensor(
        cmb_v[:], irt_a[:].rearrange("p (a b) -> p a b", b=C),
        invv_bc.to_broadcast([P, BL, C]),
        op=OP.mult)

    ssc_a = persist.tile([P, BL * C], F32, tag="ssc_a")
    nc.vector.tensor_tensor(ssc_a[:], sa_t[:],
                            cmb_a[:].rearrange("p a b -> p (a b)"), op=OP.mult)
    ssc_v = persist.tile([P, BL * C], F32, tag="ssc_v")
    nc.vector.tensor_tensor(ssc_v[:], sv_t[:],
                            cmb_v[:].rearrange("p a b -> p (a b)"), op=OP.mult)

    exp_a = persist.tile([P, BL * C], F32, tag="exp_a")
    nc.scalar.activation(exp_a[:], ssc_a[:], AF.Exp)
    exp_v = persist.tile([P, BL * C], F32, tag="exp_v")
    nc.scalar.activation(exp_v[:], ssc_v[:], AF.Exp)

    pex = psum1.tile([1, 2 * BL * C], F32, tag="pex")
    nc.tensor.matmul(out=pex[:, 0:BL * C], lhsT=ones_col[:], rhs=exp_a[:],
                     start=True, stop=True)
    nc.tensor.matmul(out=pex[:, BL * C:2 * BL * C], lhsT=ones_col[:],
                     rhs=exp_v[:], start=True, stop=True)
    pex_a = pex[:, 0:BL * C]
    pex_v = pex[:, BL * C:2 * BL * C]

    se_a = persist.tile([1, BL], F32, tag="se_a")
    nc.vector.reduce_sum(
        se_a[:], pex_a.rearrange("p (a b) -> p a b", b=C), axis=AX.X)
    se_v = persist.tile([1, BL], F32, tag="se_v")
    nc.vector.reduce_sum(
        se_v[:], pex_v.rearrange("p (a b) -> p a b", b=C), axis=AX.X)

    epos = persist.tile([1, BL], F32, tag="epos")
    nc.scalar.activation(epos[:], pos_row[:], AF.Exp)
    neg_a = persist.tile([1, BL], F32, tag="neg_a")
    nc.vector.tensor_tensor(neg_a[:], se_a[:], epos[:], op=OP.subtract)
    neg_v = persist.tile([1, BL], F32, tag="neg_v")
    nc.vector.tensor_tensor(neg_v[:], se_v[:], epos[:], op=OP.subtract)
    lg_a = persist.tile([1, BL], F32, tag="lg_a")
    nc.scalar.activation(lg_a[:], neg_a[:], AF.Ln)
    lg_v = persist.tile([1, BL], F32, tag="lg_v")
    nc.scalar.activation(lg_v[:], neg_v[:], AF.Ln)
    term = persist.tile([1, BL], F32, tag="term")
    nc.vector.tensor_tensor(term[:], lg_a[:], lg_v[:], op=OP.add)
    nc.vector.tensor_scalar_mul(term[:], term[:], 0.5)
    nc.vector.tensor_tensor(term[:], term[:], pos_row[:], op=OP.subtract)
    tot = persist.tile([1, 1], F32, tag="tot")
    nc.vector.reduce_sum(tot[:], term[:], axis=AX.X)
    nc.sync.dma_start(out[:, :], tot[:])


_CACHE = {}


def _get_nc():
    if "nc" not in _CACHE:
        nc = bass.Bass("TRN2", target_bir_lowering=False, debug=False,
                       num_devices=NCORES)
        dt = BF16 if USE_BF16 else F32
        video = nc.dram_tensor("video", [BL, T, D], dt,
                               kind="ExternalInput").ap()
        audio = nc.dram_tensor("audio", [BL, T, D], dt,
                               kind="ExternalInput").ap()
        mask = nc.dram_tensor("mask", [BL, T], I32, kind="ExternalInput").ap()
        out = nc.dram_tensor("out", [1, 1], F32, kind="ExternalOutput").ap()
        with tile.TileContext(nc) as tc:
            with ExitStack() as ctx:
                build_kernel(ctx, tc, video, audio, mask, out)
        from bir_legalize import legalize
        legalize(nc)
        _CACHE["nc"] = nc
    return _CACHE["nc"]


def kernel(video, audio, mask, _want_results=False):
    import ml_dtypes
    ddt = ml_dtypes.bfloat16 if USE_BF16 else np.float32
    video = np.ascontiguousarray(np.asarray(video).astype(ddt))
    audio = np.ascontiguousarray(np.asarray(audio).astype(ddt))
    mask = np.ascontiguousarray(np.asarray(mask, dtype=np.int32))
    nc = _get_nc()
    in_maps = []
    for i in range(NCORES):
        sl = slice(i * BL, (i + 1) * BL)
        in_maps.append({"video": video[sl], "audio": audio[sl],
                        "mask": mask[sl]})
    res = run_bass_kernel_spmd(nc, in_maps, list(range(NCORES)))
    parts = [res.results[i]["out"][0, 0] for i in range(NCORES)]
    loss = np.float32(np.sum(np.asarray(parts, dtype=np.float64)) / B)
    outarr = np.asarray([loss], dtype=np.float32)
    if _want_results:
        return outarr, res
    return outarr


# revision 11
# speedup vs baseline: 1.1280x; 1.1280x over previous
"""Contrastive loss (video/audio) Trainium2 Bass kernel.

Full inputs: video [64,512,512] f32, audio [64,512,512] f32, mask [64,512] i32.
Data-parallel over batch: 8 cores x 8 batch elements. Each core computes its
partial loss sum on device; host adds the 8 scalars and divides by B.

Per-core pipeline (v3):
  argmax(mask) via score trick -> anchor row ids -> narrow indirect gather
  [8,512] (raw anchors) -> per-b PE outer-product broadcast using an 8x8
  selector matrix (eye column broadcast as lhsT) -> PSUM [128,512] anchors.
  Main loop per b,c-chunk: r = ACT Square+accum; s_raw = DVE STT(mult,mult)
  with accum.  All normalization (sqrt/reciprocal), pos, exp, log and the
  final combine happen after the main loop; per-b anchor inverse norms are
  broadcast to 128 partitions with tiny PE outer-products and applied with a
  stride-0 free-dim AP.  One scalar partial sum is DMA'd out per core.
"""

import numpy as np
from contextlib import ExitStack

import concourse.bass as bass
import concourse.tile as tile
from concourse import mybir
from concourse.bass_utils import run_bass_kernel_spmd

F32 = mybir.dt.float32
BF16 = mybir.dt.bfloat16
I32 = mybir.dt.int32
AF = mybir.ActivationFunctionType
OP = mybir.AluOpType
AX = mybir.AxisListType

B, T, D = 64, 512, 512
NCORES = 8
BL = B // NCORES          # 8 batch elements per core
P = 128                   # partitions
C = T // P                # 4 T-chunks per matrix
TEMP = 0.07
USE_BF16 = True           # ship bf16 data; accumulate in fp32
R_ON_DVE = 11             # of the 64 r-square chunks, this many run on DVE


def build_kernel(ctx: ExitStack, tc: tile.TileContext, video, audio, mask, out):
    nc = tc.nc
    DT = BF16 if USE_BF16 else F32

    persist = ctx.enter_context(tc.tile_pool(name="persist", bufs=1))
    data = ctx.enter_context(tc.tile_pool(name="data", bufs=3))
    scr = ctx.enter_context(tc.tile_pool(name="scr", bufs=2))
    psum = ctx.enter_context(tc.tile_pool(name="psum", bufs=2, space="PSUM"))
    psum1 = ctx.enter_context(tc.tile_pool(name="psum1", bufs=1, space="PSUM"))
    dram = ctx.enter_context(tc.tile_pool(name="dram", bufs=1, space="DRAM"))

    vrows = video.rearrange("b t d -> (b t) d")     # [4096, 512]
    arows = audio.rearrange("b t d -> (b t) d")

    # ---------------- argmax(mask) -> anchor row ids -----------------------
    mask_i = persist.tile([BL, T], I32, tag="mask_i")
    nc.sync.dma_start(mask_i[:], mask[:, :])
    mask_f = persist.tile([BL, T], F32, tag="mask_f")
    nc.vector.tensor_copy(mask_f[:], mask_i[:])
    iota_i = persist.tile([BL, T], I32, tag="iota_i")
    nc.gpsimd.iota(iota_i[:], pattern=[[1, T]], base=0, channel_multiplier=0)
    iota_f = persist.tile([BL, T], F32, tag="iota_f")
    nc.vector.tensor_copy(iota_f[:], iota_i[:])
    score = persist.tile([BL, T], F32, tag="score")
    nc.vector.scalar_tensor_tensor(
        out=score[:], in0=mask_f[:], scalar=1024.0, in1=iota_f[:],
        op0=OP.mult, op1=OP.subtract)
    maxs = persist.tile([BL, 1], F32, tag="maxs")
    nc.vector.reduce_max(maxs[:], score[:], axis=AX.X)
    idx_f = persist.tile([BL, 1], F32, tag="idx_f")
    nc.vector.tensor_scalar(
        out=idx_f[:], in0=maxs[:], scalar1=-1.0, scalar2=1024.0,
        op0=OP.mult, op1=OP.add)
    nc.vector.tensor_scalar_min(idx_f[:], idx_f[:], 511.0)
    brow_i = persist.tile([BL, 1], I32, tag="brow_i")
    nc.gpsimd.iota(brow_i[:], pattern=[[1, 1]], base=0, channel_multiplier=T)
    brow_f = persist.tile([BL, 1], F32, tag="brow_f")
    nc.vector.tensor_copy(brow_f[:], brow_i[:])
    row_f = persist.tile([BL, 1], F32, tag="row_f")
    nc.vector.tensor_add(row_f[:], idx_f[:], brow_f[:])
    row_i = persist.tile([BL, 1], I32, tag="row_i")
    nc.vector.tensor_copy(row_i[:], row_f[:])

    # 8x8 selector: eye8[k,m] = (k == m), as f32
    eyei = persist.tile([BL, BL], I32, tag="eyei")
    nc.gpsimd.iota(eyei[:], pattern=[[1, BL]], base=0, channel_multiplier=-1)
    eyez = persist.tile([BL, BL], I32, tag="eyez")
    nc.vector.tensor_scalar(out=eyez[:], in0=eyei[:], scalar1=0,
                            scalar2=None, op0=OP.is_equal)
    eyef = persist.tile([BL, BL], DT, tag="eyef")
    nc.vector.tensor_copy(eyef[:], eyez[:])

    # ---------------- narrow anchor gathers: [8, 512] (raw) ----------------
    anc_v = persist.tile([BL, D], DT, tag="anc_v")
    nc.gpsimd.indirect_dma_start(
        out=anc_v[:], out_offset=None, in_=vrows[:],
        in_offset=bass.IndirectOffsetOnAxis(ap=row_i[:, :1], axis=0))
    anc_a = persist.tile([BL, D], DT, tag="anc_a")
    nc.gpsimd.indirect_dma_start(
        out=anc_a[:], out_offset=None, in_=arows[:],
        in_offset=bass.IndirectOffsetOnAxis(ap=row_i[:, :1], axis=0))

    # ---------------- main loop ---------------------------------------------
    rv_t = persist.tile([P, BL * C], F32, tag="rv_t")   # ||video_t||^2
    ra_t = persist.tile([P, BL * C], F32, tag="ra_t")   # ||audio_t||^2
    sa_t = persist.tile([P, BL * C], F32, tag="sa_t")   # video_t . anc_a(raw)
    sv_t = persist.tile([P, BL * C], F32, tag="sv_t")   # audio_t . anc_v(raw)

    # contiguous-row tiling: t = 4*p + c -> one 8KB descriptor per partition
    vid_r = video.rearrange("b (p c) d -> b p c d", p=P)   # [8,128,4,512]
    aud_r = audio.rearrange("b (p c) d -> b p c d", p=P)

    for b in range(BL):
        vt = data.tile([P, C * D], DT, tag="vid")
        nc.sync.dma_start(vt[:].rearrange("p (c d) -> p c d", d=D), vid_r[b])
        at = data.tile([P, C * D], DT, tag="aud")
        nc.sync.dma_start(at[:].rearrange("p (c d) -> p c d", d=D), aud_r[b])
        # anchor_b broadcast to all partitions: sel_b[8,128].T @ anchors[8,512]
        sel = eyef[:, b:b + 1].to_broadcast([BL, P])
        abc = psum.tile([P, D], F32, tag="abc")
        nc.tensor.matmul(out=abc[:], lhsT=sel, rhs=anc_a[:],
                         start=True, stop=True)
        vbc = psum.tile([P, D], F32, tag="vbc")
        nc.tensor.matmul(out=vbc[:], lhsT=sel, rhs=anc_v[:],
                         start=True, stop=True)
        for c in range(C):
            col = b * C + c
            vch = vt[:, c * D:(c + 1) * D]
            ach = at[:, c * D:(c + 1) * D]
            r_dve = (2 * col) % 64 < R_ON_DVE
            r1 = scr.tile([P, D], DT, tag="r1")
            if r_dve:
                nc.vector.scalar_tensor_tensor(
                    out=r1[:], in0=vch, scalar=1.0, in1=vch,
                    op0=OP.mult, op1=OP.mult,
                    accum_out=rv_t[:, col:col + 1])
            else:
                nc.scalar.activation(r1[:], vch, AF.Square,
                                     accum_out=rv_t[:, col:col + 1])
            r2 = scr.tile([P, D], DT, tag="r2")
            if (2 * col + 1) % 64 < R_ON_DVE:
                nc.vector.scalar_tensor_tensor(
                    out=r2[:], in0=ach, scalar=1.0, in1=ach,
                    op0=OP.mult, op1=OP.mult,
                    accum_out=ra_t[:, col:col + 1])
            else:
                nc.scalar.activation(r2[:], ach, AF.Square,
                                     accum_out=ra_t[:, col:col + 1])
            s1 = scr.tile([P, D], DT, tag="s1")
            nc.vector.scalar_tensor_tensor(
                out=s1[:], in0=vch, scalar=1.0, in1=abc[:],
                op0=OP.mult, op1=OP.mult, accum_out=sa_t[:, col:col + 1])
            s2 = scr.tile([P, D], DT, tag="s2")
            nc.vector.scalar_tensor_tensor(
                out=s2[:], in0=ach, scalar=1.0, in1=vbc[:],
                op0=OP.mult, op1=OP.mult, accum_out=sv_t[:, col:col + 1])

    # ------- anchor norms, pos (computed late, off the critical path) ------
    nsc_v = persist.tile([BL, D], F32, tag="nsc_v")
    ran_v = persist.tile([BL, 1], F32, tag="ran_v")
    nc.vector.scalar_tensor_tensor(
        out=nsc_v[:], in0=anc_v[:], scalar=1.0, in1=anc_v[:],
        op0=OP.mult, op1=OP.mult, accum_out=ran_v[:])
    nsc_a = persist.tile([BL, D], F32, tag="nsc_a")
    ran_a = persist.tile([BL, 1], F32, tag="ran_a")
    nc.vector.scalar_tensor_tensor(
        out=nsc_a[:], in0=anc_a[:], scalar=1.0, in1=anc_a[:],
        op0=OP.mult, op1=OP.mult, accum_out=ran_a[:])
    pd_scr = persist.tile([BL, D], F32, tag="pd_scr")
    posd = persist.tile([BL, 1], F32, tag="posd")
    nc.vector.scalar_tensor_tensor(
        out=pd_scr[:], in0=anc_v[:], scalar=1.0, in1=anc_a[:],
        op0=OP.mult, op1=OP.mult, accum_out=posd[:])
    # round-trip (ran_a, ran_v, posd) to a partition-0 row [1, 24]
    pk = persist.tile([BL, 3], F32, tag="pk")
    nc.vector.tensor_copy(pk[:, 0:1], ran_a[:])
    nc.vector.tensor_copy(pk[:, 1:2], ran_v[:])
    nc.vector.tensor_copy(pk[:, 2:3], posd[:])
    d_pk = dram.tile([BL, 3], F32, tag="d_pk")
    nc.sync.dma_start(d_pk[:], pk[:])
    pkr = persist.tile([1, BL * 3], F32, tag="pkr")
    nc.sync.dma_start(pkr[:], d_pk[:].rearrange("a b -> (a b)"))
    pkr3 = pkr[:].rearrange("p (a b) -> p a b", b=3)
    raa_row = persist.tile([1, BL], F32, tag="raa_row")
    nc.vector.tensor_copy(raa_row[:], pkr3[:, :, 0:1].rearrange("p a b -> p (a b)"))
    rav_row = persist.tile([1, BL], F32, tag="rav_row")
    nc.vector.tensor_copy(rav_row[:], pkr3[:, :, 1:2].rearrange("p a b -> p (a b)"))
    pod_row = persist.tile([1, BL], F32, tag="pod_row")
    nc.vector.tensor_copy(pod_row[:], pkr3[:, :, 2:3].rearrange("p a b -> p (a b)"))
    # inv rows: 1/(TEMP*sqrt(r))
    sq_ra = persist.tile([1, BL], F32, tag="sq_ra")
    nc.scalar.activation(sq_ra[:], raa_row[:], AF.Sqrt, scale=TEMP * TEMP)
    inva_row = persist.tile([1, BL], F32, tag="inva_row")
    nc.vector.reciprocal(inva_row[:], sq_ra[:])
    sq_rv = persist.tile([1, BL], F32, tag="sq_rv")
    nc.scalar.activation(sq_rv[:], rav_row[:], AF.Sqrt, scale=TEMP * TEMP)
    invv_row = persist.tile([1, BL], F32, tag="invv_row")
    nc.vector.reciprocal(invv_row[:], sq_rv[:])
    # pos = posd * inv_a * inv_v * TEMP   on [1,8]
    pos_row = persist.tile([1, BL], F32, tag="pos_row")
    nc.vector.tensor_tensor(pos_row[:], pod_row[:], inva_row[:], op=OP.mult)
    nc.vector.tensor_tensor(pos_row[:], pos_row[:], invv_row[:], op=OP.mult)
    nc.vector.tensor_scalar_mul(pos_row[:], pos_row[:], TEMP)
    # broadcast inv rows to [128, 8] via PE outer
    ones_row = persist.tile([1, P], F32, tag="ones_row")
    nc.vector.memset(ones_row[:], 1.0)
    ones_col = persist.tile([P, 1], F32, tag="ones_col")
    nc.vector.memset(ones_col[:], 1.0)
    inv_bc = psum1.tile([P, 2 * BL], F32, tag="inv_bc")
    nc.tensor.matmul(out=inv_bc[:, 0:BL], lhsT=ones_row[:], rhs=inva_row[:],
                     start=True, stop=True)
    nc.tensor.matmul(out=inv_bc[:, BL:2 * BL], lhsT=ones_row[:],
                     rhs=invv_row[:], start=True, stop=True)
    inva_bc = inv_bc[:, 0:BL]
    invv_bc = inv_bc[:, BL:2 * BL]

    # ---------------- post: scale, exp, reduce, combine --------------------
    srt_v = persist.tile([P, BL * C], F32, tag="srt_v")
    nc.scalar.activation(srt_v[:], rv_t[:], AF.Sqrt)
    irt_v = persist.tile([P, BL * C], F32, tag="irt_v")
    nc.vector.reciprocal(irt_v[:], srt_v[:])
    srt_a = persist.tile([P, BL * C], F32, tag="srt_a")
    nc.scalar.activation(srt_a[:], ra_t[:], AF.Sqrt)
    irt_a = persist.tile([P, BL * C], F32, tag="irt_a")
    nc.vector.reciprocal(irt_a[:], srt_a[:])

    # combined scale: irt * anchor_inv(b)  (stride-0 broadcast over c)
    cmb_a = persist.tile([P, BL, C], F32, tag="cmb_a")
    nc.vector.tensor_tensor(
        cmb_a[:], irt_v[:].rearrange("p (a b) -> p a b", b=C),
        inva_bc.to_broadcast([P, BL, C]),
        op=OP.mult)
    cmb_v = persist.tile([P, BL, C], F32, tag="cmb_v")
    nc.vector.tensor_tensor(
        cmb_v[:], irt_a[:].rearrange("p (a b) -> p a b", b=C),
        invv_bc.to_broadcast([P, BL, C]),
        op=OP.mult)

    ssc_a = persist.tile([P, BL * C], F32, tag="ssc_a")
    nc.vector.tensor_tensor(ssc_a[:], sa_t[:],
                            cmb_a[:].rearrange("p a b -> p (a b)"), op=OP.mult)
    ssc_v = persist.tile([P, BL * C], F32, tag="ssc_v")
    nc.vector.tensor_tensor(ssc_v[:], sv_t[:],
                            cmb_v[:].rearrange("p a b -> p (a b)"), op=OP.mult)

    exp_a = persist.tile([P, BL * C], F32, tag="exp_a")
    nc.scalar.activation(exp_a[:], ssc_a[:], AF.Exp)
    exp_v = persist.tile([P, BL * C], F32, tag="exp_v")
    nc.scalar.activation(exp_v[:], ssc_v[:], AF.Exp)

    pex = psum1.tile([1, 2 * BL * C], F32, tag="pex")
    nc.tensor.matmul(out=pex[:, 0:BL * C], lhsT=ones_col[:], rhs=exp_a[:],
                     start=True, stop=True)
    nc.tensor.matmul(out=pex[:, BL * C:2 * BL * C], lhsT=ones_col[:],
                     rhs=exp_v[:], start=True, stop=True)
    pex_a = pex[:, 0:BL * C]
    pex_v = pex[:, BL * C:2 * BL * C]

    se_a = persist.tile([1, BL], F32, tag="se_a")
    nc.vector.reduce_sum(
        se_a[:], pex_a.rearrange("p (a b) -> p a b", b=C), axis=AX.X)
    se_v = persist.tile([1, BL], F32, tag="se_v")
    nc.vector.reduce_sum(
        se_v[:], pex_v.rearrange("p (a b) -> p a b", b=C), axis=AX.X)

    epos = persist.tile([1, BL], F32, tag="epos")
    nc.scalar.activation(epos[:], pos_row[:], AF.Exp)
    neg_a = persist.tile([1, BL], F32, tag="neg_a")
    nc.vector.tensor_tensor(neg_a[:], se_a[:], epos[:], op=OP.subtract)
    neg_v = persist.tile([1, BL], F32, tag="neg_v")
    nc.vector.tensor_tensor(neg_v[:], se_v[:], epos[:], op=OP.subtract)
    lg_a = persist.tile([1, BL], F32, tag="lg_a")
    nc.scalar.activation(lg_a[:], neg_a[:], AF.Ln)
    lg_v = persist.tile([1, BL], F32, tag="lg_v")
    nc.scalar.activation(lg_v[:], neg_v[:], AF.Ln)
    term = persist.tile([1, BL], F32, tag="term")
    nc.vector.tensor_tensor(term[:], lg_a[:], lg_v[:], op=OP.add)
    nc.vector.tensor_scalar_mul(term[:], term[:], 0.5)
    nc.vector.tensor_tensor(term[:], term[:], pos_row[:], op=OP.subtract)
    tot = persist.tile([1, 1], F32, tag="tot")
    nc.vector.reduce_sum(tot[:], term[:], axis=AX.X)
    nc.sync.dma_start(out[:, :], tot[:])


_CACHE = {}


def _get_nc():
    if "nc" not in _CACHE:
        nc = bass.Bass("TRN2", target_bir_lowering=False, debug=False,
                       num_devices=NCORES)
        dt = BF16 if USE_BF16 else F32
        video = nc.dram_tensor("video", [BL, T, D], dt,
                               kind="ExternalInput").ap()
        audio = nc.dram_tensor("audio", [BL, T, D], dt,
                               kind="ExternalInput").ap()
        mask = nc.dram_tensor("mask", [BL, T], I32, kind="ExternalInput").ap()
        out = nc.dram_tensor("out", [1, 1], F32, kind="ExternalOutput").ap()
        with tile.TileContext(nc) as tc:
            with ExitStack() as ctx:
                build_kernel(ctx, tc, video, audio, mask, out)
        from bir_legalize import legalize
        legalize(nc)
        _CACHE["nc"] = nc
    return _CACHE["nc"]


def kernel(video, audio, mask, _want_results=False):
    import ml_dtypes
    ddt = ml_dtypes.bfloat16 if USE_BF16 else np.float32
    video = np.ascontiguousarray(np.asarray(video).astype(ddt))
    audio = np.ascontiguousarray(np.asarray(audio).astype(ddt))
    mask = np.ascontiguousarray(np.asarray(mask, dtype=np.int32))
    nc = _get_nc()
    in_maps = []
    for i in range(NCORES):
        sl = slice(i * BL, (i + 1) * BL)
        in_maps.append({"video": video[sl], "audio": audio[sl],
                        "mask": mask[sl]})
    res = run_bass_kernel_spmd(nc, in_maps, list(range(NCORES)))
    parts = [res.results[i]["out"][0, 0] for i in range(NCORES)]
    loss = np.float32(np.sum(np.asarray(parts, dtype=np.float64)) / B)
    outarr = np.asarray([loss], dtype=np.float32)
    if _want_results:
        return outarr, res
    return outarr


# revision 13
# speedup vs baseline: 1.1600x; 1.0284x over previous
"""Contrastive loss (video/audio) Trainium2 Bass kernel.

Full inputs: video [64,512,512] f32, audio [64,512,512] f32, mask [64,512] i32.
Data-parallel over batch: 8 cores x 8 batch elements. Each core computes its
partial loss sum on device; host adds the 8 scalars and divides by B.

Per-core pipeline (v5, bf16 data / fp32 accumulation):
  mask -> score -> one-hot indicator (natural layout) -> PE-transposed to
  T-partitioned layout.  Anchor rows are extracted AND broadcast in one step:
  abc[b] = sum_c (ind_t[:,(c,b)] bcast as lhsT) @ chunk_c  -- 4 accumulating
  matmuls per (b, modality), PSUM [128,512].  No indirect DMA.
  Main loop per (b,c): r = row sum-of-squares (ACT Square+accum, a slice on
  DVE STT for balance); s_raw = DVE STT(chunk * abc, accum).
  Anchor norms and the positive logit are recovered from the r/s accumulator
  tiles with the same indicator (elementwise mult + PE ones-matvec), so the
  whole normalization/exp/log tail runs on [1,8] partition-0 rows.
"""

import numpy as np
from contextlib import ExitStack

import concourse.bass as bass
import concourse.tile as tile
from concourse import mybir
from concourse.bass_utils import run_bass_kernel_spmd

F32 = mybir.dt.float32
BF16 = mybir.dt.bfloat16
I32 = mybir.dt.int32
AF = mybir.ActivationFunctionType
OP = mybir.AluOpType
AX = mybir.AxisListType

B, T, D = 64, 512, 512
NCORES = 8
BL = B // NCORES          # 8 batch elements per core
P = 128                   # partitions
C = T // P                # 4 T-chunks per matrix
TEMP = 0.07
USE_BF16 = True
R_ON_DVE = 10             # of the 64 r-square chunks, this many run on DVE


def build_kernel(ctx: ExitStack, tc: tile.TileContext, video, audio, mask, out):
    nc = tc.nc
    DT = BF16 if USE_BF16 else F32

    persist = ctx.enter_context(tc.tile_pool(name="persist", bufs=1))
    data = ctx.enter_context(tc.tile_pool(name="data", bufs=3))
    scr = ctx.enter_context(tc.tile_pool(name="scr", bufs=2))
    psum = ctx.enter_context(tc.tile_pool(name="psum", bufs=2, space="PSUM"))
    psum1 = ctx.enter_context(tc.tile_pool(name="psum1", bufs=1, space="PSUM"))

    # ---------------- data loads (issue first: mask, then b tiles) ---------
    mask_i = persist.tile([BL, T], I32, tag="mask_i")
    nc.sync.dma_start(mask_i[:], mask[:, :])

    # block tiling: t = c*128 + p  (matches PE-transposed indicator blocks)
    vid_r = video.rearrange("b (c p) d -> b p c d", p=P)   # [8,128,4,512]
    aud_r = audio.rearrange("b (c p) d -> b p c d", p=P)
    vts, ats = [], []
    for b in range(BL):
        vt = data.tile([P, C * D], DT, tag="vid")
        nc.sync.dma_start(vt[:].rearrange("p (c d) -> p c d", d=D), vid_r[b])
        at = data.tile([P, C * D], DT, tag="aud")
        nc.sync.dma_start(at[:].rearrange("p (c d) -> p c d", d=D), aud_r[b])
        vts.append(vt)
        ats.append(at)

    # ---------------- indicator: natural then T-partitioned ----------------
    mask_f = persist.tile([BL, T], F32, tag="mask_f")
    nc.vector.tensor_copy(mask_f[:], mask_i[:])
    iota_i = persist.tile([BL, T], I32, tag="iota_i")
    nc.gpsimd.iota(iota_i[:], pattern=[[1, T]], base=0, channel_multiplier=0)
    iota_f = persist.tile([BL, T], F32, tag="iota_f")
    nc.vector.tensor_copy(iota_f[:], iota_i[:])
    score = persist.tile([BL, T], F32, tag="score")
    nc.vector.scalar_tensor_tensor(
        out=score[:], in0=mask_f[:], scalar=1024.0, in1=iota_f[:],
        op0=OP.mult, op1=OP.subtract)
    maxs = persist.tile([BL, 1], F32, tag="maxs")
    nc.vector.reduce_max(maxs[:], score[:], axis=AX.X)
    ind_nat = persist.tile([BL, T], DT, tag="ind_nat")   # one-hot, exact 0/1
    nc.vector.tensor_scalar(out=ind_nat[:], in0=score[:],
                            scalar1=maxs[:, :1], scalar2=None,
                            op0=OP.is_equal)
    # 8x8 identity for the PE transpose
    eyei = persist.tile([BL, BL], I32, tag="eyei")
    nc.gpsimd.iota(eyei[:], pattern=[[1, BL]], base=0, channel_multiplier=-1)
    eyez = persist.tile([BL, BL], I32, tag="eyez")
    nc.vector.tensor_scalar(out=eyez[:], in0=eyei[:], scalar1=0,
                            scalar2=None, op0=OP.is_equal)
    eyef = persist.tile([BL, BL], DT, tag="eyef")
    nc.vector.tensor_copy(eyef[:], eyez[:])
    # transpose each [8,128] block -> [128,8]; ind_t cols are (c, b)
    ind_t = persist.tile([P, C * BL], DT, tag="ind_t")
    for c in range(C):
        tp = psum1.tile([P, BL], DT, tag="tp")
        nc.tensor.transpose(out=tp[:], in_=ind_nat[:, c * P:(c + 1) * P],
                            identity=eyef[:])
        nc.vector.tensor_copy(ind_t[:, c * BL:c * BL + BL], tp[:])

    # ---------------- main loop ---------------------------------------------
    rv_t = persist.tile([P, BL * C], F32, tag="rv_t")   # ||video_t||^2
    ra_t = persist.tile([P, BL * C], F32, tag="ra_t")   # ||audio_t||^2
    sa_t = persist.tile([P, BL * C], F32, tag="sa_t")   # video_t . anc_a(raw)
    sv_t = persist.tile([P, BL * C], F32, tag="sv_t")   # audio_t . anc_v(raw)

    for b in range(BL):
        vt, at = vts[b], ats[b]
        # anchor extraction fused with broadcast:
        # abc[m,n] = sum_c sum_p ind_t[p,(c,b)] * audio[t(c,p), n]
        abc = psum.tile([P, D], F32, tag="abc")
        vbc = psum.tile([P, D], F32, tag="vbc")
        for c in range(C):
            sel = ind_t[:, c * BL + b:c * BL + b + 1].to_broadcast([P, P])
            nc.tensor.matmul(out=abc[:], lhsT=sel,
                             rhs=at[:, c * D:(c + 1) * D],
                             start=(c == 0), stop=(c == C - 1))
        for c in range(C):
            sel = ind_t[:, c * BL + b:c * BL + b + 1].to_broadcast([P, P])
            nc.tensor.matmul(out=vbc[:], lhsT=sel,
                             rhs=vt[:, c * D:(c + 1) * D],
                             start=(c == 0), stop=(c == C - 1))
        # r first (keeps ACT/DVE streams unblocked), then s
        for c in range(C):
            col = b * C + c
            vch = vt[:, c * D:(c + 1) * D]
            ach = at[:, c * D:(c + 1) * D]
            r1 = scr.tile([P, D], DT, tag="r1")
            if (2 * col) % 64 < R_ON_DVE:
                nc.vector.scalar_tensor_tensor(
                    out=r1[:], in0=vch, scalar=1.0, in1=vch,
                    op0=OP.mult, op1=OP.mult,
                    accum_out=rv_t[:, col:col + 1])
            else:
                nc.scalar.activation(r1[:], vch, AF.Square,
                                     accum_out=rv_t[:, col:col + 1])
            r2 = scr.tile([P, D], DT, tag="r2")
            if (2 * col + 1) % 64 < R_ON_DVE:
                nc.vector.scalar_tensor_tensor(
                    out=r2[:], in0=ach, scalar=1.0, in1=ach,
                    op0=OP.mult, op1=OP.mult,
                    accum_out=ra_t[:, col:col + 1])
            else:
                nc.scalar.activation(r2[:], ach, AF.Square,
                                     accum_out=ra_t[:, col:col + 1])
        for c in range(C):
            col = b * C + c
            vch = vt[:, c * D:(c + 1) * D]
            ach = at[:, c * D:(c + 1) * D]
            s1 = scr.tile([P, D], DT, tag="s1")
            nc.vector.scalar_tensor_tensor(
                out=s1[:], in0=vch, scalar=1.0, in1=abc[:],
                op0=OP.mult, op1=OP.mult, accum_out=sa_t[:, col:col + 1])
            s2 = scr.tile([P, D], DT, tag="s2")
            nc.vector.scalar_tensor_tensor(
                out=s2[:], in0=ach, scalar=1.0, in1=vbc[:],
                op0=OP.mult, op1=OP.mult, accum_out=sv_t[:, col:col + 1])

    # ------- anchor norms + pos from accumulator tiles via indicator -------
    # ind_t cols are (c,b); accumulator cols are (b,c): use a strided view.
    ind_bc = ind_t[:].rearrange("p (c b) -> p c b", b=BL) \
        .rearrange("p c b -> p b c")                       # [128, b, c] view
    prod = persist.tile([P, 3 * BL * C], F32, tag="prod")
    pv = prod[:, 0:BL * C].rearrange("p (b c) -> p b c", c=C)
    pa = prod[:, BL * C:2 * BL * C].rearrange("p (b c) -> p b c", c=C)
    pp = prod[:, 2 * BL * C:3 * BL * C].rearrange("p (b c) -> p b c", c=C)
    nc.vector.tensor_tensor(pv, rv_t[:].rearrange("p (b c) -> p b c", c=C),
                            ind_bc, op=OP.mult)
    nc.vector.tensor_tensor(pa, ra_t[:].rearrange("p (b c) -> p b c", c=C),
                            ind_bc, op=OP.mult)
    nc.vector.tensor_tensor(pp, sa_t[:].rearrange("p (b c) -> p b c", c=C),
                            ind_bc, op=OP.mult)
    ones_col = persist.tile([P, 1], F32, tag="ones_col")
    nc.vector.memset(ones_col[:], 1.0)
    psel = psum1.tile([1, 3 * BL * C], F32, tag="psel")
    nc.tensor.matmul(out=psel[:], lhsT=ones_col[:], rhs=prod[:],
                     start=True, stop=True)
    rav_row = persist.tile([1, BL], F32, tag="rav_row")   # ||anc_v||^2
    nc.vector.reduce_sum(
        rav_row[:], psel[:, 0:BL * C].rearrange("p (b c) -> p b c", c=C),
        axis=AX.X)
    raa_row = persist.tile([1, BL], F32, tag="raa_row")   # ||anc_a||^2
    nc.vector.reduce_sum(
        raa_row[:],
        psel[:, BL * C:2 * BL * C].rearrange("p (b c) -> p b c", c=C),
        axis=AX.X)
    pod_row = persist.tile([1, BL], F32, tag="pod_row")   # anc_v . anc_a
    nc.vector.reduce_sum(
        pod_row[:],
        psel[:, 2 * BL * C:3 * BL * C].rearrange("p (b c) -> p b c", c=C),
        axis=AX.X)
    sq_ra = persist.tile([1, BL], F32, tag="sq_ra")
    nc.scalar.activation(sq_ra[:], raa_row[:], AF.Sqrt, scale=TEMP * TEMP)
    inva_row = persist.tile([1, BL], F32, tag="inva_row")
    nc.vector.reciprocal(inva_row[:], sq_ra[:])
    sq_rv = persist.tile([1, BL], F32, tag="sq_rv")
    nc.scalar.activation(sq_rv[:], rav_row[:], AF.Sqrt, scale=TEMP * TEMP)
    invv_row = persist.tile([1, BL], F32, tag="invv_row")
    nc.vector.reciprocal(invv_row[:], sq_rv[:])
    pos_row = persist.tile([1, BL], F32, tag="pos_row")
    nc.vector.tensor_tensor(pos_row[:], pod_row[:], inva_row[:], op=OP.mult)
    nc.vector.tensor_tensor(pos_row[:], pos_row[:], invv_row[:], op=OP.mult)
    nc.vector.tensor_scalar_mul(pos_row[:], pos_row[:], TEMP)
    # broadcast inv rows to [128, 8] via PE outer
    ones_row = persist.tile([1, P], F32, tag="ones_row")
    nc.vector.memset(ones_row[:], 1.0)
    inv_bc = psum1.tile([P, 2 * BL], F32, tag="inv_bc")
    nc.tensor.matmul(out=inv_bc[:, 0:BL], lhsT=ones_row[:], rhs=inva_row[:],
                     start=True, stop=True)
    nc.tensor.matmul(out=inv_bc[:, BL:2 * BL], lhsT=ones_row[:],
                     rhs=invv_row[:], start=True, stop=True)
    inva_bc = inv_bc[:, 0:BL]
    invv_bc = inv_bc[:, BL:2 * BL]

    # ---------------- post: scale, exp, reduce, combine --------------------
    srt_v = persist.tile([P, BL * C], F32, tag="srt_v")
    nc.scalar.activation(srt_v[:], rv_t[:], AF.Sqrt)
    irt_v = persist.tile([P, BL * C], F32, tag="irt_v")
    nc.vector.reciprocal(irt_v[:], srt_v[:])
    srt_a = persist.tile([P, BL * C], F32, tag="srt_a")
    nc.scalar.activation(srt_a[:], ra_t[:], AF.Sqrt)
    irt_a = persist.tile([P, BL * C], F32, tag="irt_a")
    nc.vector.reciprocal(irt_a[:], srt_a[:])

    cmb_a = persist.tile([P, BL, C], F32, tag="cmb_a")
    nc.vector.tensor_tensor(
        cmb_a[:], irt_v[:].rearrange("p (a b) -> p a b", b=C),
        inva_bc.to_broadcast([P, BL, C]), op=OP.mult)
    cmb_v = persist.tile([P, BL, C], F32, tag="cmb_v")
    nc.vector.tensor_tensor(
        cmb_v[:], irt_a[:].rearrange("p (a b) -> p a b", b=C),
        invv_bc.to_broadcast([P, BL, C]), op=OP.mult)

    ssc_a = persist.tile([P, BL * C], F32, tag="ssc_a")
    nc.vector.tensor_tensor(ssc_a[:], sa_t[:],
                            cmb_a[:].rearrange("p a b -> p (a b)"), op=OP.mult)
    ssc_v = persist.tile([P, BL * C], F32, tag="ssc_v")
    nc.vector.tensor_tensor(ssc_v[:], sv_t[:],
                            cmb_v[:].rearrange("p a b -> p (a b)"), op=OP.mult)

    exp_a = persist.tile([P, BL * C], F32, tag="exp_a")
    nc.scalar.activation(exp_a[:], ssc_a[:], AF.Exp)
    exp_v = persist.tile([P, BL * C], F32, tag="exp_v")
    nc.scalar.activation(exp_v[:], ssc_v[:], AF.Exp)

    pex = psum1.tile([1, 2 * BL * C], F32, tag="pex")
    nc.tensor.matmul(out=pex[:, 0:BL * C], lhsT=ones_col[:], rhs=exp_a[:],
                     start=True, stop=True)
    nc.tensor.matmul(out=pex[:, BL * C:2 * BL * C], lhsT=ones_col[:],
                     rhs=exp_v[:], start=True, stop=True)

    se_a = persist.tile([1, BL], F32, tag="se_a")
    nc.vector.reduce_sum(
        se_a[:], pex[:, 0:BL * C].rearrange("p (a b) -> p a b", b=C),
        axis=AX.X)
    se_v = persist.tile([1, BL], F32, tag="se_v")
    nc.vector.reduce_sum(
        se_v[:], pex[:, BL * C:2 * BL * C].rearrange("p (a b) -> p a b", b=C),
        axis=AX.X)

    epos = persist.tile([1, BL], F32, tag="epos")
    nc.scalar.activation(epos[:], pos_row[:], AF.Exp)
    neg_a = persist.tile([1, BL], F32, tag="neg_a")
    nc.vector.tensor_tensor(neg_a[:], se_a[:], epos[:], op=OP.subtract)
    neg_v = persist.tile([1, BL], F32, tag="neg_v")
    nc.vector.tensor_tensor(neg_v[:], se_v[:], epos[:], op=OP.subtract)
    lg_a = persist.tile([1, BL], F32, tag="lg_a")
    nc.scalar.activation(lg_a[:], neg_a[:], AF.Ln)
    lg_v = persist.tile([1, BL], F32, tag="lg_v")
    nc.scalar.activation(lg_v[:], neg_v[:], AF.Ln)
    term = persist.tile([1, BL], F32, tag="term")
    nc.vector.tensor_tensor(term[:], lg_a[:], lg_v[:], op=OP.add)
    nc.vector.tensor_scalar_mul(term[:], term[:], 0.5)
    nc.vector.tensor_tensor(term[:], term[:], pos_row[:], op=OP.subtract)
    tot = persist.tile([1, 1], F32, tag="tot")
    nc.vector.reduce_sum(tot[:], term[:], axis=AX.X)
    nc.sync.dma_start(out[:, :], tot[:])


_CACHE = {}


def _get_nc():
    if "nc" not in _CACHE:
        nc = bass.Bass("TRN2", target_bir_lowering=False, debug=False,
                       num_devices=NCORES)
        dt = BF16 if USE_BF16 else F32
        video = nc.dram_tensor("video", [BL, T, D], dt,
                               kind="ExternalInput").ap()
        audio = nc.dram_tensor("audio", [BL, T, D], dt,
                               kind="ExternalInput").ap()
        mask = nc.dram_tensor("mask", [BL, T], I32, kind="ExternalInput").ap()
        out = nc.dram_tensor("out", [1, 1], F32, kind="ExternalOutput").ap()
        with tile.TileContext(nc) as tc:
            with ExitStack() as ctx:
                build_kernel(ctx, tc, video, audio, mask, out)
        from bir_legalize import legalize
        legalize(nc)
        _CACHE["nc"] = nc
    return _CACHE["nc"]


def kernel(video, audio, mask, _want_results=False):
    import ml_dtypes
    ddt = ml_dtypes.bfloat16 if USE_BF16 else np.float32
    video = np.ascontiguousarray(np.asarray(video).astype(ddt))
    audio = np.ascontiguousarray(np.asarray(audio).astype(ddt))
    mask = np.ascontiguousarray(np.asarray(mask, dtype=np.int32))
    nc = _get_nc()
    in_maps = []
    for i in range(NCORES):
        sl = slice(i * BL, (i + 1) * BL)
        in_maps.append({"video": video[sl], "audio": audio[sl],
                        "mask": mask[sl]})
    res = run_bass_kernel_spmd(nc, in_maps, list(range(NCORES)))
    parts = [res.results[i]["out"][0, 0] for i in range(NCORES)]
    loss = np.float32(np.sum(np.asarray(parts, dtype=np.float64)) / B)
    outarr = np.asarray([loss], dtype=np.float32)
    if _want_results:
        return outarr, res
    return outarr


# revision 14
# speedup vs baseline: 1.2534x; 1.0806x over previous
"""Contrastive loss (video/audio) Trainium2 Bass kernel.

Full inputs: video [64,512,512] f32, audio [64,512,512] f32, mask [64,512] i32.
Data-parallel over batch: 8 cores x 8 batch elements. Each core computes its
partial loss sum on device; host adds the 8 scalars and divides by B.

Per-core pipeline (v5, bf16 data / fp32 accumulation):
  mask -> score -> one-hot indicator (natural layout) -> PE-transposed to
  T-partitioned layout.  Anchor rows are extracted AND broadcast in one step:
  abc[b] = sum_c (ind_t[:,(c,b)] bcast as lhsT) @ chunk_c  -- 4 accumulating
  matmuls per (b, modality), PSUM [128,512].  No indirect DMA.
  Main loop per (b,c): r = row sum-of-squares (ACT Square+accum, a slice on
  DVE STT for balance); s_raw = DVE STT(chunk * abc, accum).
  Anchor norms and the positive logit are recovered from the r/s accumulator
  tiles with the same indicator (elementwise mult + PE ones-matvec), so the
  whole normalization/exp/log tail runs on [1,8] partition-0 rows.
"""

import numpy as np
from contextlib import ExitStack

import concourse.bass as bass
import concourse.tile as tile
from concourse import mybir
from concourse.bass_utils import run_bass_kernel_spmd

F32 = mybir.dt.float32
BF16 = mybir.dt.bfloat16
I32 = mybir.dt.int32
AF = mybir.ActivationFunctionType
OP = mybir.AluOpType
AX = mybir.AxisListType

B, T, D = 64, 512, 512
NCORES = 8
BL = B // NCORES          # 8 batch elements per core
P = 128                   # partitions
C = T // P                # 4 T-chunks per matrix
TEMP = 0.07
USE_BF16 = True
R_ON_DVE = 10             # of the 64 r-square chunks, this many run on DVE


def build_kernel(ctx: ExitStack, tc: tile.TileContext, video, audio, mask, out):
    nc = tc.nc
    DT = BF16 if USE_BF16 else F32

    persist = ctx.enter_context(tc.tile_pool(name="persist", bufs=1))
    data = ctx.enter_context(tc.tile_pool(name="data", bufs=3))
    scr = ctx.enter_context(tc.tile_pool(name="scr", bufs=2))
    psum = ctx.enter_context(tc.tile_pool(name="psum", bufs=2, space="PSUM"))
    psum1 = ctx.enter_context(tc.tile_pool(name="psum1", bufs=1, space="PSUM"))

    # ---------------- data loads (issue first: mask, then b tiles) ---------
    mask_i = persist.tile([BL, T], I32, tag="mask_i")
    nc.sync.dma_start(mask_i[:], mask[:, :])

    # block tiling: t = c*128 + p  (matches PE-transposed indicator blocks)
    vid_r = video.rearrange("b (c p) d -> b p c d", p=P)   # [8,128,4,512]
    aud_r = audio.rearrange("b (c p) d -> b p c d", p=P)
    vts, ats = [], []
    for b in range(BL):
        vt = data.tile([P, C * D], DT, tag="vid")
        nc.sync.dma_start(vt[:].rearrange("p (c d) -> p c d", d=D), vid_r[b])
        at = data.tile([P, C * D], DT, tag="aud")
        nc.sync.dma_start(at[:].rearrange("p (c d) -> p c d", d=D), aud_r[b])
        vts.append(vt)
        ats.append(at)

    # ---------------- indicator: natural then T-partitioned ----------------
    mask_f = persist.tile([BL, T], F32, tag="mask_f")
    nc.vector.tensor_copy(mask_f[:], mask_i[:])
    iota_i = persist.tile([BL, T], I32, tag="iota_i")
    nc.gpsimd.iota(iota_i[:], pattern=[[1, T]], base=0, channel_multiplier=0)
    iota_f = persist.tile([BL, T], F32, tag="iota_f")
    nc.vector.tensor_copy(iota_f[:], iota_i[:])
    score = persist.tile([BL, T], F32, tag="score")
    nc.vector.scalar_tensor_tensor(
        out=score[:], in0=mask_f[:], scalar=1024.0, in1=iota_f[:],
        op0=OP.mult, op1=OP.subtract)
    maxs = persist.tile([BL, 1], F32, tag="maxs")
    nc.vector.reduce_max(maxs[:], score[:], axis=AX.X)
    ind_nat = persist.tile([BL, T], DT, tag="ind_nat")   # one-hot, exact 0/1
    nc.vector.tensor_scalar(out=ind_nat[:], in0=score[:],
                            scalar1=maxs[:, :1], scalar2=None,
                            op0=OP.is_equal)
    # 8x8 identity for the PE transpose
    eyei = persist.tile([BL, BL], I32, tag="eyei")
    nc.gpsimd.iota(eyei[:], pattern=[[1, BL]], base=0, channel_multiplier=-1)
    eyez = persist.tile([BL, BL], I32, tag="eyez")
    nc.vector.tensor_scalar(out=eyez[:], in0=eyei[:], scalar1=0,
                            scalar2=None, op0=OP.is_equal)
    eyef = persist.tile([BL, BL], DT, tag="eyef")
    nc.vector.tensor_copy(eyef[:], eyez[:])
    # transpose each [8,128] block -> [128,8]; ind_t cols are (c, b)
    ind_t = persist.tile([P, C * BL], DT, tag="ind_t")
    for c in range(C):
        tp = psum1.tile([P, BL], DT, tag="tp")
        nc.tensor.transpose(out=tp[:], in_=ind_nat[:, c * P:(c + 1) * P],
                            identity=eyef[:])
        nc.vector.tensor_copy(ind_t[:, c * BL:c * BL + BL], tp[:])

    # ---------------- main loop ---------------------------------------------
    rv_t = persist.tile([P, BL * C], F32, tag="rv_t")   # ||video_t||^2
    ra_t = persist.tile([P, BL * C], F32, tag="ra_t")   # ||audio_t||^2
    sa_t = persist.tile([P, BL * C], F32, tag="sa_t")   # video_t . anc_a(raw)
    sv_t = persist.tile([P, BL * C], F32, tag="sv_t")   # audio_t . anc_v(raw)

    for b in range(BL):
        vt, at = vts[b], ats[b]
        # anchor extraction fused with broadcast:
        # abc[m,n] = sum_c sum_p ind_t[p,(c,b)] * audio[t(c,p), n]
        abc = psum.tile([P, D], F32, tag="abc")
        vbc = psum.tile([P, D], F32, tag="vbc")
        for c in range(C):
            sel = ind_t[:, c * BL + b:c * BL + b + 1].to_broadcast([P, P])
            nc.tensor.matmul(out=abc[:], lhsT=sel,
                             rhs=at[:, c * D:(c + 1) * D],
                             start=(c == 0), stop=(c == C - 1))
        for c in range(C):
            sel = ind_t[:, c * BL + b:c * BL + b + 1].to_broadcast([P, P])
            nc.tensor.matmul(out=vbc[:], lhsT=sel,
                             rhs=vt[:, c * D:(c + 1) * D],
                             start=(c == 0), stop=(c == C - 1))
        # r first (keeps ACT/DVE streams unblocked), then s
        for c in range(C):
            col = b * C + c
            vch = vt[:, c * D:(c + 1) * D]
            ach = at[:, c * D:(c + 1) * D]
            if (2 * col) % 64 < R_ON_DVE:
                r1 = scr.tile([P, D], DT, tag="r1d")
                nc.vector.scalar_tensor_tensor(
                    out=r1[:], in0=vch, scalar=1.0, in1=vch,
                    op0=OP.mult, op1=OP.mult,
                    accum_out=rv_t[:, col:col + 1])
            else:
                r1 = scr.tile([P, D], DT, tag="r1a")
                nc.scalar.activation(r1[:], vch, AF.Square,
                                     accum_out=rv_t[:, col:col + 1])
            if (2 * col + 1) % 64 < R_ON_DVE:
                r2 = scr.tile([P, D], DT, tag="r2d")
                nc.vector.scalar_tensor_tensor(
                    out=r2[:], in0=ach, scalar=1.0, in1=ach,
                    op0=OP.mult, op1=OP.mult,
                    accum_out=ra_t[:, col:col + 1])
            else:
                r2 = scr.tile([P, D], DT, tag="r2a")
                nc.scalar.activation(r2[:], ach, AF.Square,
                                     accum_out=ra_t[:, col:col + 1])
        for c in range(C):
            col = b * C + c
            vch = vt[:, c * D:(c + 1) * D]
            ach = at[:, c * D:(c + 1) * D]
            s1 = scr.tile([P, D], DT, tag="s1")
            nc.vector.scalar_tensor_tensor(
                out=s1[:], in0=vch, scalar=1.0, in1=abc[:],
                op0=OP.mult, op1=OP.mult, accum_out=sa_t[:, col:col + 1])
            s2 = scr.tile([P, D], DT, tag="s2")
            nc.vector.scalar_tensor_tensor(
                out=s2[:], in0=ach, scalar=1.0, in1=vbc[:],
                op0=OP.mult, op1=OP.mult, accum_out=sv_t[:, col:col + 1])

    # ------- anchor norms + pos from accumulator tiles via indicator -------
    # ind_t cols are (c,b); accumulator cols are (b,c): use a strided view.
    ind_bc = ind_t[:].rearrange("p (c b) -> p c b", b=BL) \
        .rearrange("p c b -> p b c")                       # [128, b, c] view
    prod = persist.tile([P, 3 * BL * C], F32, tag="prod")
    pv = prod[:, 0:BL * C].rearrange("p (b c) -> p b c", c=C)
    pa = prod[:, BL * C:2 * BL * C].rearrange("p (b c) -> p b c", c=C)
    pp = prod[:, 2 * BL * C:3 * BL * C].rearrange("p (b c) -> p b c", c=C)
    nc.vector.tensor_tensor(pv, rv_t[:].rearrange("p (b c) -> p b c", c=C),
                            ind_bc, op=OP.mult)
    nc.vector.tensor_tensor(pa, ra_t[:].rearrange("p (b c) -> p b c", c=C),
                            ind_bc, op=OP.mult)
    nc.vector.tensor_tensor(pp, sa_t[:].rearrange("p (b c) -> p b c", c=C),
                            ind_bc, op=OP.mult)
    ones_col = persist.tile([P, 1], F32, tag="ones_col")
    nc.vector.memset(ones_col[:], 1.0)
    psel = psum1.tile([1, 3 * BL * C], F32, tag="psel")
    nc.tensor.matmul(out=psel[:], lhsT=ones_col[:], rhs=prod[:],
                     start=True, stop=True)
    rav_row = persist.tile([1, BL], F32, tag="rav_row")   # ||anc_v||^2
    nc.vector.reduce_sum(
        rav_row[:], psel[:, 0:BL * C].rearrange("p (b c) -> p b c", c=C),
        axis=AX.X)
    raa_row = persist.tile([1, BL], F32, tag="raa_row")   # ||anc_a||^2
    nc.vector.reduce_sum(
        raa_row[:],
        psel[:, BL * C:2 * BL * C].rearrange("p (b c) -> p b c", c=C),
        axis=AX.X)
    pod_row = persist.tile([1, BL], F32, tag="pod_row")   # anc_v . anc_a
    nc.vector.reduce_sum(
        pod_row[:],
        psel[:, 2 * BL * C:3 * BL * C].rearrange("p (b c) -> p b c", c=C),
        axis=AX.X)
    sq_ra = persist.tile([1, BL], F32, tag="sq_ra")
    nc.scalar.activation(sq_ra[:], raa_row[:], AF.Sqrt, scale=TEMP * TEMP)
    inva_row = persist.tile([1, BL], F32, tag="inva_row")
    nc.vector.reciprocal(inva_row[:], sq_ra[:])
    sq_rv = persist.tile([1, BL], F32, tag="sq_rv")
    nc.scalar.activation(sq_rv[:], rav_row[:], AF.Sqrt, scale=TEMP * TEMP)
    invv_row = persist.tile([1, BL], F32, tag="invv_row")
    nc.vector.reciprocal(invv_row[:], sq_rv[:])
    pos_row = persist.tile([1, BL], F32, tag="pos_row")
    nc.vector.tensor_tensor(pos_row[:], pod_row[:], inva_row[:], op=OP.mult)
    nc.vector.tensor_tensor(pos_row[:], pos_row[:], invv_row[:], op=OP.mult)
    nc.vector.tensor_scalar_mul(pos_row[:], pos_row[:], TEMP)
    # broadcast inv rows to [128, 8] via PE outer
    ones_row = persist.tile([1, P], F32, tag="ones_row")
    nc.vector.memset(ones_row[:], 1.0)
    inv_bc = psum1.tile([P, 2 * BL], F32, tag="inv_bc")
    nc.tensor.matmul(out=inv_bc[:, 0:BL], lhsT=ones_row[:], rhs=inva_row[:],
                     start=True, stop=True)
    nc.tensor.matmul(out=inv_bc[:, BL:2 * BL], lhsT=ones_row[:],
                     rhs=invv_row[:], start=True, stop=True)
    inva_bc = inv_bc[:, 0:BL]
    invv_bc = inv_bc[:, BL:2 * BL]

    # ---------------- post: scale, exp, reduce, combine --------------------
    srt_v = persist.tile([P, BL * C], F32, tag="srt_v")
    nc.scalar.activation(srt_v[:], rv_t[:], AF.Sqrt)
    irt_v = persist.tile([P, BL * C], F32, tag="irt_v")
    nc.vector.reciprocal(irt_v[:], srt_v[:])
    srt_a = persist.tile([P, BL * C], F32, tag="srt_a")
    nc.scalar.activation(srt_a[:], ra_t[:], AF.Sqrt)
    irt_a = persist.tile([P, BL * C], F32, tag="irt_a")
    nc.vector.reciprocal(irt_a[:], srt_a[:])

    cmb_a = persist.tile([P, BL, C], F32, tag="cmb_a")
    nc.vector.tensor_tensor(
        cmb_a[:], irt_v[:].rearrange("p (a b) -> p a b", b=C),
        inva_bc.to_broadcast([P, BL, C]), op=OP.mult)
    cmb_v = persist.tile([P, BL, C], F32, tag="cmb_v")
    nc.vector.tensor_tensor(
        cmb_v[:], irt_a[:].rearrange("p (a b) -> p a b", b=C),
        invv_bc.to_broadcast([P, BL, C]), op=OP.mult)

    ssc_a = persist.tile([P, BL * C], F32, tag="ssc_a")
    nc.vector.tensor_tensor(ssc_a[:], sa_t[:],
                            cmb_a[:].rearrange("p a b -> p (a b)"), op=OP.mult)
    ssc_v = persist.tile([P, BL * C], F32, tag="ssc_v")
    nc.vector.tensor_tensor(ssc_v[:], sv_t[:],
                            cmb_v[:].rearrange("p a b -> p (a b)"), op=OP.mult)

    exp_a = persist.tile([P, BL * C], F32, tag="exp_a")
    nc.scalar.activation(exp_a[:], ssc_a[:], AF.Exp)
    exp_v = persist.tile([P, BL * C], F32, tag="exp_v")
    nc.scalar.activation(exp_v[:], ssc_v[:], AF.Exp)

    pex = psum1.tile([1, 2 * BL * C], F32, tag="pex")
    nc.tensor.matmul(out=pex[:, 0:BL * C], lhsT=ones_col[:], rhs=exp_a[:],
                     start=True, stop=True)
    nc.tensor.matmul(out=pex[:, BL * C:2 * BL * C], lhsT=ones_col[:],
                     rhs=exp_v[:], start=True, stop=True)

    se_a = persist.tile([1, BL], F32, tag="se_a")
    nc.vector.reduce_sum(
        se_a[:], pex[:, 0:BL * C].rearrange("p (a b) -> p a b", b=C),
        axis=AX.X)
    se_v = persist.tile([1, BL], F32, tag="se_v")
    nc.vector.reduce_sum(
        se_v[:], pex[:, BL * C:2 * BL * C].rearrange("p (a b) -> p a b", b=C),
        axis=AX.X)

    epos = persist.tile([1, BL], F32, tag="epos")
    nc.scalar.activation(epos[:], pos_row[:], AF.Exp)
    neg_a = persist.tile([1, BL], F32, tag="neg_a")
    nc.vector.tensor_tensor(neg_a[:], se_a[:], epos[:], op=OP.subtract)
    neg_v = persist.tile([1, BL], F32, tag="neg_v")
    nc.vector.tensor_tensor(neg_v[:], se_v[:], epos[:], op=OP.subtract)
    lg_a = persist.tile([1, BL], F32, tag="lg_a")
    nc.scalar.activation(lg_a[:], neg_a[:], AF.Ln)
    lg_v = persist.tile([1, BL], F32, tag="lg_v")
    nc.scalar.activation(lg_v[:], neg_v[:], AF.Ln)
    term = persist.tile([1, BL], F32, tag="term")
    nc.vector.tensor_tensor(term[:], lg_a[:], lg_v[:], op=OP.add)
    nc.vector.tensor_scalar_mul(term[:], term[:], 0.5)
    nc.vector.tensor_tensor(term[:], term[:], pos_row[:], op=OP.subtract)
    tot = persist.tile([1, 1], F32, tag="tot")
    nc.vector.reduce_sum(tot[:], term[:], axis=AX.X)
    nc.sync.dma_start(out[:, :], tot[:])


_CACHE = {}


def _get_nc():
    if "nc" not in _CACHE:
        nc = bass.Bass("TRN2", target_bir_lowering=False, debug=False,
                       num_devices=NCORES)
        dt = BF16 if USE_BF16 else F32
        video = nc.dram_tensor("video", [BL, T, D], dt,
                               kind="ExternalInput").ap()
        audio = nc.dram_tensor("audio", [BL, T, D], dt,
                               kind="ExternalInput").ap()
        mask = nc.dram_tensor("mask", [BL, T], I32, kind="ExternalInput").ap()
        out = nc.dram_tensor("out", [1, 1], F32, kind="ExternalOutput").ap()
        with tile.TileContext(nc) as tc:
            with ExitStack() as ctx:
                build_kernel(ctx, tc, video, audio, mask, out)
        from bir_legalize import legalize
        legalize(nc)
        _CACHE["nc"] = nc
    return _CACHE["nc"]


def kernel(video, audio, mask, _want_results=False):
    import ml_dtypes
    ddt = ml_dtypes.bfloat16 if USE_BF16 else np.float32
    video = np.ascontiguousarray(np.asarray(video).astype(ddt))
    audio = np.ascontiguousarray(np.asarray(audio).astype(ddt))
    mask = np.ascontiguousarray(np.asarray(mask, dtype=np.int32))
    nc = _get_nc()
    in_maps = []
    for i in range(NCORES):
        sl = slice(i * BL, (i + 1) * BL)
        in_maps.append({"video": video[sl], "audio": audio[sl],
                        "mask": mask[sl]})
    res = run_bass_kernel_spmd(nc, in_maps, list(range(NCORES)))
    parts = [res.results[i]["out"][0, 0] for i in range(NCORES)]
    loss = np.float32(np.sum(np.asarray(parts, dtype=np.float64)) / B)
    outarr = np.asarray([loss], dtype=np.float32)
    if _want_results:
        return outarr, res
    return outarr
